# revision 10
# baseline (speedup 1.0000x reference)
"""Channel-attention kernel for Trainium2 (8 NeuronCores, SPMD data-parallel).

out[b] = beta * softmax(rowmax(S) - S, axis=-1) @ x[b] + x[b],  S = x[b] @ x[b].T

Sharding: batch dim B=16 split as 2 batches per core across 8 cores.

The kernel is DMA-bound (measured ~286 GB/s/core effective with all 8
cores running), so HBM traffic is minimized:
  - x is loaded as a host-precast bf16 copy (4 MB/batch instead of 8).
  - xT is a host-pretransposed fp8 copy (2 MB/batch) for the S matmul,
    whose contraction dim n must live on SBUF partitions.
  - out is stored as bf16 (4 MB/batch) and upcast to fp32 on the host.
Total 10 MB/batch vs 18 MB for the fp32 variant.

The S matmul runs fp8 with perf_mode=DoubleRow (K=256/instruction).

The second matmul (feat = A^T.T @ x) runs fp8 DoubleRow into 2-bank
[128, 1024] PSUM tiles; the residual out = feat + x is one wide DVE
tensor_add per tile pair (fp32 PSUM + bf16 x -> bf16 out), so the x
term's precision is bf16 (~2^-9 relative).

Math note: softmax(max_row - S) row-wise equals exp(minrow - S) / Z with
Z = sum_d exp(minrow - S).  beta/Z is folded into A before the second
matmul, so when beta == 0 the output is exactly bf16(x).
"""

from contextlib import ExitStack

import numpy as np
import ml_dtypes

N_CORES = 8
B, C, N = 16, 512, 4096
BPC = B // N_CORES  # batches per core
P = 128
MT = C // P  # 4 row-blocks of channels
KT = N // P  # 32 partition-tiles of xT
XT_CH = 2  # xT dma/dep chunks (16 k-tiles each, 1 MB per DMA)
NQ = N // 512  # 8 n-chunks for the second matmul
KD = C // P  # 4 d-chunks for the second matmul
XG = 2  # x/out tiles per batch (2 m-blocks each, 2 MB per DMA)

# drain engine per feat psum tile (16 per batch): a=ACT copy (with +x
# folded in PSUM via identity matmul), v=DVE tensor_add.
# Measured: any ACT share of the drains loses (all-DVE 91us; 4a 111.8us;
# 7a 114.9us) — ACT drains delay the next batch's casts in ACT's
# in-order queue.
DRAIN_PLAN = "vvvvvvvvvvvvvvvv"

_CACHE = {}


def _build_bass(reps=1, loop_iters=1, dma_only=False, stage=99):
    # stage: 0=dma only, 1=+S matmul, 2=+softmax, 3=+transposes, >=4 full
    if dma_only:
        stage = 0
    import concourse.bass as bass
    import concourse.bacc as bacc
    import concourse.mybir as mybir
    from concourse import tile, masks

    dt = mybir.dt
    AF = mybir.ActivationFunctionType
    ALU = mybir.AluOpType
    AX = mybir.AxisListType
    DR = mybir.MatmulPerfMode.DoubleRow

    nc = bacc.Bacc(
        "TRN2", target_bir_lowering=False, debug=False, num_devices=N_CORES
    )

    x_dram = nc.dram_tensor("x", [BPC, C, N], dt.bfloat16, kind="ExternalInput")
    # xt is host-pre-shuffled to the exact SBUF tile layout
    # [chunk, partition, k_local, c] so each chunk loads as one fully
    # contiguous 1 MB DMA.
    xt_dram = nc.dram_tensor(
        "xt", [BPC, XT_CH, P, KT // XT_CH, C], dt.float8e4, kind="ExternalInput"
    )
    beta_dram = nc.dram_tensor("beta", [1, 1], dt.float32, kind="ExternalInput")
    out_dram = nc.dram_tensor("out", [BPC, C, N], dt.bfloat16, kind="ExternalOutput")

    with tile.TileContext(nc) as tc, ExitStack() as ctx:
        const_pool = ctx.enter_context(tc.tile_pool(name="const", bufs=1))
        x_pool = ctx.enter_context(tc.tile_pool(name="x", bufs=2 * XG))
        xt_pool = ctx.enter_context(tc.tile_pool(name="xt", bufs=2 * XT_CH))
        xb_pool = ctx.enter_context(tc.tile_pool(name="xb", bufs=3))
        o_pool = ctx.enter_context(tc.tile_pool(name="o", bufs=3))
        a_pool = ctx.enter_context(tc.tile_pool(name="a", bufs=2))
        at_pool = ctx.enter_context(tc.tile_pool(name="at", bufs=2))
        st_pool = ctx.enter_context(tc.tile_pool(name="st", bufs=2))
        spsum = ctx.enter_context(
            tc.tile_pool(name="spsum", bufs=3, space=bass.MemorySpace.PSUM)
        )
        tpsum = ctx.enter_context(
            tc.tile_pool(name="tpsum", bufs=1, space=bass.MemorySpace.PSUM)
        )
        fpsum = ctx.enter_context(
            tc.tile_pool(name="fpsum", bufs=2, space=bass.MemorySpace.PSUM)
        )

        ident = const_pool.tile([P, P], dt.bfloat16)
        masks.make_identity(nc, ident[:])

        # Broadcast beta scalar to all 128 partitions via ones.T @ beta.
        ones = const_pool.tile([1, P], dt.float32)
        nc.gpsimd.memset(ones[:], 1.0)
        beta_sb = const_pool.tile([1, 1], dt.float32)
        nc.sync.dma_start(beta_sb[:], beta_dram[:])
        beta_ps = spsum.tile([P, 1], dt.float32, tag="s_ps")
        nc.tensor.matmul(beta_ps[:], ones[:], beta_sb[:], start=True, stop=True)
        beta128 = const_pool.tile([P, 1], dt.float32)
        nc.scalar.copy(beta128[:], beta_ps[:])

        def emit_batch(b):
            # ---- loads ----
            xtt = []
            for ch in range(XT_CH):
                t = xt_pool.tile([P, KT // XT_CH, C], dt.float8e4, tag="xtt")
                nc.sync.dma_start(t[:], xt_dram[b, ch])
                xtt.append(t)
            # x as XG tiles of 2 m-blocks each: c = 128*(2t+u) + p
            x_src = x_dram[b].rearrange("(t u p) n -> t p u n", p=P, u=2)
            x_tiles = []
            for t in range(XG):
                xt_t = x_pool.tile([P, 2, N], dt.bfloat16, tag="x")
                nc.sync.dma_start(xt_t[:], x_src[t])
                x_tiles.append(xt_t)

            def xm(j):
                # view of x m-block j: [128, N] bf16
                return x_tiles[j // 2][:, j % 2]

            out_dst = out_dram[b].rearrange("(t u p) n -> t p u n", p=P, u=2)

            if stage == 0:
                for t in range(XG):
                    nc.sync.dma_start(out_dst[t], x_tiles[t][:])
                return

            # ---- S = x @ x.T  (fp8 DoubleRow, K=256 per instruction) ----
            s_tiles = []
            for m in range(MT):
                s_ps = spsum.tile([P, 512], dt.float32, tag="s_ps")
                for ch in range(XT_CH):
                    for h in range(KT // XT_CH // 2):
                        nc.tensor.matmul(
                            s_ps[:],
                            xtt[ch][:, 2 * h : 2 * h + 2, P * m : P * (m + 1)],
                            xtt[ch][:, 2 * h : 2 * h + 2, :],
                            start=(ch == 0 and h == 0),
                            stop=(ch == XT_CH - 1 and h == KT // XT_CH // 2 - 1),
                            perf_mode=DR,
                        )
                s_tiles.append(s_ps)

            if stage == 1:
                for t in range(XG):
                    nc.sync.dma_start(out_dst[t], x_tiles[t][:])
                return

            # ---- cast x -> fp8 for the F matmul's moving operand ----
            # (all ACT: DVE paces the kernel at ~69us/shard vs the 62us
            # DMA floor, while ACT has ~26us slack)
            xb_tiles = []
            for t in range(XG):
                xb_t = xb_pool.tile([P, 2, N], dt.float8e4, tag="xb")
                if t == 0:
                    nc.scalar.copy(xb_t[:], x_tiles[t][:])
                else:
                    nc.scalar.copy(xb_t[:, 0], x_tiles[t][:, 0])
                    nc.vector.tensor_copy(xb_t[:, 1], x_tiles[t][:, 1])
                xb_tiles.append(xb_t)

            # ---- softmax: A = exp(minrow - S) * (beta / Z) ----
            a_sb = a_pool.tile([P, MT, 512], dt.bfloat16)
            minr = st_pool.tile([P, MT], dt.float32, tag="minr")
            zsum = st_pool.tile([P, MT], dt.float32, tag="z")
            rzb = st_pool.tile([P, MT], dt.float32, tag="rzb")
            for m in range(MT):
                nc.vector.tensor_reduce(
                    minr[:, m : m + 1], s_tiles[m][:], axis=AX.X, op=ALU.min
                )
                nc.scalar.activation(
                    a_sb[:, m, :],
                    s_tiles[m][:],
                    AF.Exp,
                    bias=minr[:, m : m + 1],
                    scale=-1.0,
                    accum_out=zsum[:, m : m + 1],
                )
                nc.vector.reciprocal(rzb[:, m : m + 1], zsum[:, m : m + 1])
                nc.vector.tensor_mul(
                    rzb[:, m : m + 1], rzb[:, m : m + 1], beta128[:]
                )
                nc.vector.tensor_scalar_mul(
                    a_sb[:, m, :], a_sb[:, m, :], rzb[:, m : m + 1]
                )

            if stage == 2:
                for t in range(XG):
                    nc.sync.dma_start(out_dst[t], x_tiles[t][:])
                return

            # ---- A^T = (beta/Z * A)^T as fp8, per 128x128 block (PE) ----
            # 4 transposes of row-block m land in one PSUM bank; one big
            # copy moves them to at_sb (at_sb[:, j, Pm:P(m+1)] = A^T block
            # (j, m), so the bank holds [j, 128] contiguous per m).
            at_sb = at_pool.tile([P, KD, 512], dt.float8e4)
            for m in range(MT):
                t_ps = tpsum.tile([P, KD, P], dt.bfloat16, tag="t_ps")
                for j in range(KD):
                    nc.tensor.transpose(
                        t_ps[:, j], a_sb[:, m, P * j : P * (j + 1)], ident[:]
                    )
                if m % 2 == 0:
                    nc.scalar.copy(at_sb[:, :, P * m : P * (m + 1)], t_ps[:])
                else:
                    nc.vector.tensor_copy(
                        at_sb[:, :, P * m : P * (m + 1)], t_ps[:]
                    )

            if stage == 3:
                for t in range(XG):
                    nc.sync.dma_start(out_dst[t], x_tiles[t][:])
                return

            # ---- feat = A^T.T @ x (fp8 DoubleRow into fp32 PSUM),
            #      out = feat + x, split across ACT and DVE: 'a' tiles
            #      get +x folded in PSUM via an identity bf16 matmul so
            #      ACT drains with a plain copy; 'v' tiles tensor_add on
            #      DVE.  (All-DVE drains left DVE ~6 us/shard over the
            #      62 us DMA floor.) ----
            ti = 0
            for t in range(XG):
                o_t = o_pool.tile([P, 2, N], dt.bfloat16, tag="o")
                for u in range(2):
                    m = 2 * t + u
                    for q2 in range(NQ // 2):
                        dr = DRAIN_PLAN[ti % len(DRAIN_PLAN)]
                        ti += 1
                        f_ps = fpsum.tile([P, 1024], dt.float32, tag="f_ps")
                        for h in range(2):
                            q = 2 * q2 + h
                            for v in range(2):
                                nc.tensor.matmul(
                                    f_ps[:, 512 * h : 512 * (h + 1)],
                                    at_sb[:, 2 * v : 2 * v + 2, P * m : P * (m + 1)],
                                    xb_tiles[v][:, :, 512 * q : 512 * (q + 1)],
                                    start=(v == 0),
                                    stop=(v == 1 and dr != "a"),
                                    perf_mode=DR,
                                )
                            if dr == "a":
                                nc.tensor.matmul(
                                    f_ps[:, 512 * h : 512 * (h + 1)],
                                    ident[:],
                                    xm(m)[:, 512 * q : 512 * (q + 1)],
                                    start=False,
                                    stop=True,
                                )
                        dst = o_t[:, u, 1024 * q2 : 1024 * (q2 + 1)]
                        if dr == "a":
                            nc.scalar.copy(dst, f_ps[:])
                        else:
                            nc.vector.tensor_add(
                                dst, f_ps[:], xm(m)[:, 1024 * q2 : 1024 * (q2 + 1)]
                            )
                nc.sync.dma_start(out_dst[t], o_t[:])

        def emit_rep():
            for b in range(BPC):
                emit_batch(b)

        if loop_iters > 1:
            with tc.For_i(0, loop_iters, 1):
                for _ in range(reps):
                    emit_rep()
        else:
            for _ in range(reps):
                emit_rep()

    nc.compile()
    return nc


def _get_nc(reps=1, loop_iters=1, dma_only=False, stage=99):
    key = ("nc", reps, loop_iters, dma_only, stage)
    if key not in _CACHE:
        _CACHE[key] = _build_bass(reps, loop_iters, dma_only, stage)
    return _CACHE[key]


def _make_in_maps(x, beta):
    x = np.ascontiguousarray(x, dtype=np.float32)
    xbf = x.astype(ml_dtypes.bfloat16)
    xt8 = np.ascontiguousarray(
        x.transpose(0, 2, 1), dtype=np.float32
    ).astype(ml_dtypes.float8_e4m3)
    # reorder to the kernel's SBUF tile layout: [b, ch, p, k_local, c]
    # where n = 128 * ((KT//XT_CH)*ch + k_local) + p
    xt8 = np.ascontiguousarray(
        xt8.reshape(B, XT_CH, KT // XT_CH, P, C).transpose(0, 1, 3, 2, 4)
    )
    beta_arr = np.asarray(beta, dtype=np.float32).reshape(1, 1)
    in_maps = []
    for i in range(N_CORES):
        sl = slice(BPC * i, BPC * (i + 1))
        in_maps.append(
            {
                "x": np.ascontiguousarray(xbf[sl]),
                "xt": np.ascontiguousarray(xt8[sl]),
                "beta": beta_arr,
            }
        )
    return in_maps


def _run(x, beta, trace=False, **kwargs):
    from concourse.bass_utils import run_bass_kernel_spmd

    nc = _get_nc()
    in_maps = _make_in_maps(x, beta)
    res = run_bass_kernel_spmd(
        nc, in_maps, core_ids=list(range(N_CORES)), trace=trace, **kwargs
    )
    out = np.concatenate([np.asarray(r["out"]) for r in res.results], axis=0)
    return out.astype(np.float32, copy=False), res


def kernel(x, beta):
    out, _ = _run(np.asarray(x), np.asarray(beta))
    return out



# revision 13
# speedup vs baseline: 1.0987x; 1.0987x over previous
"""Channel-attention kernel for Trainium2 (8 NeuronCores, SPMD data-parallel).

out[b] = beta * softmax(rowmax(S) - S, axis=-1) @ x[b] + x[b],  S = x[b] @ x[b].T

Sharding: batch dim B=16 split as 2 batches per core across 8 cores.

The kernel is DMA-bound (measured ~286 GB/s/core effective with all 8
cores running), so HBM traffic is minimized:
  - x is loaded as a host-precast bf16 copy (4 MB/batch instead of 8).
  - xT is a host-pretransposed fp8 copy (2 MB/batch) for the S matmul,
    whose contraction dim n must live on SBUF partitions.
  - out is stored as bf16 (4 MB/batch) and upcast to fp32 on the host.
Total 10 MB/batch vs 18 MB for the fp32 variant.

The S matmul runs fp8 with perf_mode=DoubleRow (K=256/instruction).

The second matmul (feat = A^T.T @ x) runs fp8 DoubleRow into 2-bank
[128, 1024] PSUM tiles; the residual out = feat + x is one wide DVE
tensor_add per tile pair (fp32 PSUM + bf16 x -> bf16 out), so the x
term's precision is bf16 (~2^-9 relative).

Math note: softmax(max_row - S) row-wise equals exp(minrow - S) / Z with
Z = sum_d exp(minrow - S).  beta/Z is folded into A before the second
matmul, so when beta == 0 the output is exactly bf16(x).
"""

from contextlib import ExitStack

import numpy as np
import ml_dtypes

N_CORES = 8
B, C, N = 16, 512, 4096
BPC = B // N_CORES  # batches per core
P = 128
MT = C // P  # 4 row-blocks of channels
KT = N // P  # 32 partition-tiles of xT
XT_CH = 2  # xT dma/dep chunks (16 k-tiles each, 1 MB per DMA)
NQ = N // 512  # 8 n-chunks for the second matmul
KD = C // P  # 4 d-chunks for the second matmul
XG = 2  # x/out tiles per batch (2 m-blocks each, 2 MB per DMA)

# drain engine per feat psum tile (16 per batch): a=ACT copy (with +x
# folded in PSUM via identity matmul), v=DVE tensor_add.
# Measured: any ACT share of the drains loses (all-DVE 91us; 4a 111.8us;
# 7a 114.9us) — ACT drains delay the next batch's casts in ACT's
# in-order queue.
DRAIN_PLAN = "vvvvvvvvvvvvvvvv"

_CACHE = {}


def _build_bass(reps=1, loop_iters=1, dma_only=False, stage=99):
    # stage: 0=dma only, 1=+S matmul, 2=+softmax, 3=+transposes, >=4 full
    if dma_only:
        stage = 0
    import concourse.bass as bass
    import concourse.bacc as bacc
    import concourse.mybir as mybir
    from concourse import tile, masks

    dt = mybir.dt
    AF = mybir.ActivationFunctionType
    ALU = mybir.AluOpType
    AX = mybir.AxisListType
    DR = mybir.MatmulPerfMode.DoubleRow

    nc = bacc.Bacc(
        "TRN2", target_bir_lowering=False, debug=False, num_devices=N_CORES
    )

    x_dram = nc.dram_tensor("x", [BPC, C, N], dt.bfloat16, kind="ExternalInput")
    # xt is host-pre-shuffled to the exact SBUF tile layout
    # [chunk, partition, k_local, c] so each chunk loads as one fully
    # contiguous 1 MB DMA.
    xt_dram = nc.dram_tensor(
        "xt", [BPC, XT_CH, P, KT // XT_CH, C], dt.float8e4, kind="ExternalInput"
    )
    beta_dram = nc.dram_tensor("beta", [1, 1], dt.float32, kind="ExternalInput")
    out_dram = nc.dram_tensor("out", [BPC, C, N], dt.bfloat16, kind="ExternalOutput")

    with tile.TileContext(nc) as tc, ExitStack() as ctx:
        const_pool = ctx.enter_context(tc.tile_pool(name="const", bufs=1))
        x_pool = ctx.enter_context(tc.tile_pool(name="x", bufs=2 * XG))
        xt_pool = ctx.enter_context(tc.tile_pool(name="xt", bufs=2 * XT_CH))
        xb_pool = ctx.enter_context(tc.tile_pool(name="xb", bufs=3))
        o_pool = ctx.enter_context(tc.tile_pool(name="o", bufs=3))
        a_pool = ctx.enter_context(tc.tile_pool(name="a", bufs=2))
        at_pool = ctx.enter_context(tc.tile_pool(name="at", bufs=2))
        st_pool = ctx.enter_context(tc.tile_pool(name="st", bufs=2))
        spsum = ctx.enter_context(
            tc.tile_pool(name="spsum", bufs=3, space=bass.MemorySpace.PSUM)
        )
        tpsum = ctx.enter_context(
            tc.tile_pool(name="tpsum", bufs=1, space=bass.MemorySpace.PSUM)
        )
        fpsum = ctx.enter_context(
            tc.tile_pool(name="fpsum", bufs=2, space=bass.MemorySpace.PSUM)
        )

        ident = const_pool.tile([P, P], dt.bfloat16)
        masks.make_identity(nc, ident[:])

        # Broadcast beta scalar to all 128 partitions via ones.T @ beta.
        ones = const_pool.tile([1, P], dt.float32)
        nc.gpsimd.memset(ones[:], 1.0)
        beta_sb = const_pool.tile([1, 1], dt.float32)
        nc.sync.dma_start(beta_sb[:], beta_dram[:])
        beta_ps = spsum.tile([P, 1], dt.float32, tag="s_ps")
        nc.tensor.matmul(beta_ps[:], ones[:], beta_sb[:], start=True, stop=True)
        beta128 = const_pool.tile([P, 1], dt.float32)
        nc.scalar.copy(beta128[:], beta_ps[:])

        def emit_batch(b):
            # ---- loads ----
            xtt = []
            for ch in range(XT_CH):
                t = xt_pool.tile([P, KT // XT_CH, C], dt.float8e4, tag="xtt")
                nc.sync.dma_start(t[:], xt_dram[b, ch])
                xtt.append(t)
            # x as XG tiles of 2 m-blocks each: c = 128*(2t+u) + p
            x_src = x_dram[b].rearrange("(t u p) n -> t p u n", p=P, u=2)
            x_tiles = []
            for t in range(XG):
                xt_t = x_pool.tile([P, 2, N], dt.bfloat16, tag="x")
                nc.sync.dma_start(xt_t[:], x_src[t])
                x_tiles.append(xt_t)

            def xm(j):
                # view of x m-block j: [128, N] bf16
                return x_tiles[j // 2][:, j % 2]

            out_dst = out_dram[b].rearrange("(t u p) n -> t p u n", p=P, u=2)

            if stage == 0:
                for t in range(XG):
                    nc.sync.dma_start(out_dst[t], x_tiles[t][:])
                return

            # ---- S = x @ x.T  (fp8 DoubleRow, K=256 per instruction) ----
            s_tiles = []
            for m in range(MT):
                s_ps = spsum.tile([P, 512], dt.float32, tag="s_ps")
                for ch in range(XT_CH):
                    for h in range(KT // XT_CH // 2):
                        nc.tensor.matmul(
                            s_ps[:],
                            xtt[ch][:, 2 * h : 2 * h + 2, P * m : P * (m + 1)],
                            xtt[ch][:, 2 * h : 2 * h + 2, :],
                            start=(ch == 0 and h == 0),
                            stop=(ch == XT_CH - 1 and h == KT // XT_CH // 2 - 1),
                            perf_mode=DR,
                        )
                s_tiles.append(s_ps)

            if stage == 1:
                for t in range(XG):
                    nc.sync.dma_start(out_dst[t], x_tiles[t][:])
                return

            # ---- cast x -> fp8 for the F matmul's moving operand ----
            # (all ACT: DVE paces the kernel at ~69us/shard vs the 62us
            # DMA floor, while ACT has ~26us slack)
            xb_tiles = []
            for t in range(XG):
                xb_t = xb_pool.tile([P, 2, N], dt.float8e4, tag="xb")
                if t == 0:
                    nc.scalar.copy(xb_t[:], x_tiles[t][:])
                else:
                    nc.scalar.copy(xb_t[:, 0], x_tiles[t][:, 0])
                    nc.vector.tensor_copy(xb_t[:, 1], x_tiles[t][:, 1])
                xb_tiles.append(xb_t)

            # ---- softmax: A = exp(minrow - S) * (beta / Z) ----
            a_sb = a_pool.tile([P, MT, 512], dt.bfloat16)
            minr = st_pool.tile([P, MT], dt.float32, tag="minr")
            zsum = st_pool.tile([P, MT], dt.float32, tag="z")
            rzb = st_pool.tile([P, MT], dt.float32, tag="rzb")
            for m in range(MT):
                nc.vector.tensor_reduce(
                    minr[:, m : m + 1], s_tiles[m][:], axis=AX.X, op=ALU.min
                )
                nc.scalar.activation(
                    a_sb[:, m, :],
                    s_tiles[m][:],
                    AF.Exp,
                    bias=minr[:, m : m + 1],
                    scale=-1.0,
                    accum_out=zsum[:, m : m + 1],
                )
                nc.vector.reciprocal(rzb[:, m : m + 1], zsum[:, m : m + 1])
                nc.vector.tensor_mul(
                    rzb[:, m : m + 1], rzb[:, m : m + 1], beta128[:]
                )
                nc.vector.tensor_scalar_mul(
                    a_sb[:, m, :], a_sb[:, m, :], rzb[:, m : m + 1]
                )

            if stage == 2:
                for t in range(XG):
                    nc.sync.dma_start(out_dst[t], x_tiles[t][:])
                return

            # ---- A^T = (beta/Z * A)^T as fp8, per 128x128 block (PE) ----
            # 4 transposes of row-block m land in one PSUM bank; one big
            # copy moves them to at_sb (at_sb[:, j, Pm:P(m+1)] = A^T block
            # (j, m), so the bank holds [j, 128] contiguous per m).
            at_sb = at_pool.tile([P, KD, 512], dt.float8e4)
            for m in range(MT):
                t_ps = tpsum.tile([P, KD, P], dt.bfloat16, tag="t_ps")
                for j in range(KD):
                    nc.tensor.transpose(
                        t_ps[:, j], a_sb[:, m, P * j : P * (j + 1)], ident[:]
                    )
                if m % 2 == 0:
                    nc.scalar.copy(at_sb[:, :, P * m : P * (m + 1)], t_ps[:])
                else:
                    nc.vector.tensor_copy(
                        at_sb[:, :, P * m : P * (m + 1)], t_ps[:]
                    )

            if stage == 3:
                for t in range(XG):
                    nc.sync.dma_start(out_dst[t], x_tiles[t][:])
                return

            # ---- feat = A^T.T @ x (fp8 DoubleRow into fp32 PSUM),
            #      out = feat + x, split across ACT and DVE: 'a' tiles
            #      get +x folded in PSUM via an identity bf16 matmul so
            #      ACT drains with a plain copy; 'v' tiles tensor_add on
            #      DVE.  (All-DVE drains left DVE ~6 us/shard over the
            #      62 us DMA floor.) ----
            ti = 0
            for t in range(XG):
                o_t = o_pool.tile([P, 2, N], dt.bfloat16, tag="o")
                for u in range(2):
                    m = 2 * t + u
                    for q2 in range(NQ // 2):
                        dr = DRAIN_PLAN[ti % len(DRAIN_PLAN)]
                        ti += 1
                        f_ps = fpsum.tile([P, 1024], dt.float32, tag="f_ps")
                        for h in range(2):
                            q = 2 * q2 + h
                            for v in range(2):
                                nc.tensor.matmul(
                                    f_ps[:, 512 * h : 512 * (h + 1)],
                                    at_sb[:, 2 * v : 2 * v + 2, P * m : P * (m + 1)],
                                    xb_tiles[v][:, :, 512 * q : 512 * (q + 1)],
                                    start=(v == 0),
                                    stop=(v == 1 and dr != "a"),
                                    perf_mode=DR,
                                )
                            if dr == "a":
                                nc.tensor.matmul(
                                    f_ps[:, 512 * h : 512 * (h + 1)],
                                    ident[:],
                                    xm(m)[:, 512 * q : 512 * (q + 1)],
                                    start=False,
                                    stop=True,
                                )
                        dst = o_t[:, u, 1024 * q2 : 1024 * (q2 + 1)]
                        if dr == "a":
                            nc.scalar.copy(dst, f_ps[:])
                        else:
                            nc.vector.tensor_add(
                                dst, f_ps[:], xm(m)[:, 1024 * q2 : 1024 * (q2 + 1)]
                            )
                nc.sync.dma_start(out_dst[t], o_t[:])

        def emit_rep():
            for b in range(BPC):
                emit_batch(b)

        if loop_iters > 1:
            with tc.For_i(0, loop_iters, 1):
                for _ in range(reps):
                    emit_rep()
        else:
            for _ in range(reps):
                emit_rep()

    nc.compile()
    return nc


def _get_nc(reps=1, loop_iters=1, dma_only=False, stage=99):
    key = ("nc", reps, loop_iters, dma_only, stage)
    if key not in _CACHE:
        _CACHE[key] = _build_bass(reps, loop_iters, dma_only, stage)
    return _CACHE[key]


def _make_in_maps(x, beta):
    x = np.ascontiguousarray(x, dtype=np.float32)
    xbf = x.astype(ml_dtypes.bfloat16)
    xt8 = np.ascontiguousarray(
        x.transpose(0, 2, 1), dtype=np.float32
    ).astype(ml_dtypes.float8_e4m3)
    # reorder to the kernel's SBUF tile layout: [b, ch, p, k_local, c]
    # where n = 128 * ((KT//XT_CH)*ch + k_local) + p
    xt8 = np.ascontiguousarray(
        xt8.reshape(B, XT_CH, KT // XT_CH, P, C).transpose(0, 1, 3, 2, 4)
    )
    beta_arr = np.asarray(beta, dtype=np.float32).reshape(1, 1)
    in_maps = []
    for i in range(N_CORES):
        sl = slice(BPC * i, BPC * (i + 1))
        in_maps.append(
            {
                "x": np.ascontiguousarray(xbf[sl]),
                "xt": np.ascontiguousarray(xt8[sl]),
                "beta": beta_arr,
            }
        )
    return in_maps


def _run(x, beta, trace=False, **kwargs):
    from concourse.bass_utils import run_bass_kernel_spmd

    nc = _get_nc()
    in_maps = _make_in_maps(x, beta)
    res = run_bass_kernel_spmd(
        nc, in_maps, core_ids=list(range(N_CORES)), trace=trace, **kwargs
    )
    out = np.concatenate([np.asarray(r["out"]) for r in res.results], axis=0)
    return out.astype(np.float32, copy=False), res


def kernel(x, beta):
    out, _ = _run(np.asarray(x), np.asarray(beta))
    return out

